# revision 1
# baseline (speedup 1.0000x reference)
"""Trainium2 Bass kernel: softmax(catid_time_matrix) row-gather (embedding lookup).

reference:
    probs = softmax(catid_time_matrix, axis=1)   # [168, 2048] fp32
    out   = probs[inputs_hour]                   # [512, 200, 2048] fp32

Strategy (8 NeuronCores, data-parallel over batch):
  - Each core handles 64 batches = 12800 tokens; the [168, 2048] table is
    replicated and softmaxed on-chip.
  - The output is 12800 copies (per core) of 168 distinct 8 KB rows that
    live in SBUF after the softmax.  The device issues indirect
    scatter-DMAs: one instruction writes, for each SBUF partition p, the
    table row it holds straight to a dynamic DRAM row offset (up to 128
    rows = 1 MB per instruction).  Unused lanes carry an out-of-bounds
    sentinel which the DMA bounds-check skips.
  - 168 slots > 128 partitions, so L=4 rotated copies of the softmaxed
    table are built in SBUF (layout j: partition p holds slot
    (p + b_j) % 168).  The host wrapper packs token positions round-robin
    over the rotations so nearly every instruction uses all 128 lanes,
    which keeps all 16 SDMA engines busy and balanced (~142 instructions
    instead of 208 half-empty ones).
  - HBM traffic is write-only (~105 MB/core) - the memory roofline.
  - Raw bass (no Tile) so the scatters carry no artificial write-after-
    write dependencies; completion is guaranteed by a trailing flush DMA
    on the same SWDGE queue (per-engine rings drain in order).
"""

import numpy as np

import concourse.bass as bass
import concourse.mybir as mybir
from concourse import bacc
from concourse.bass_utils import run_bass_kernel_spmd

NUM_SLOTS = 168
NUM_CATS = 2048
BATCH, SEQ = 512, 200
N_CORES = 8
B_CORE = BATCH // N_CORES       # 64 batches per core
TOK = B_CORE * SEQ              # 12800 tokens per core
P = 128
PAD_SLOTS = 2 * P               # table input padded to 256 rows host-side
ROTS = (0, 42, 84, 126)         # layout j: partition p holds slot (p+b_j)%168
L = len(ROTS)
OOB = np.int32(2**31 - 2)       # > bounds_check -> row silently skipped

f32 = mybir.dt.float32
i32 = mybir.dt.int32


def _rotation_pieces(b):
    """Contiguous (src_chunk, src_lo, dst_lo, n) pieces building the rotated
    layout: dst partition p holds slot (p+b)%168, sourced from probs0
    (slots 0..127) and probs1 (slots 128..167 on partitions 0..39)."""
    pieces = []
    p = 0
    while p < P:
        s = (p + b) % NUM_SLOTS
        if s < 128:
            n = min(P - p, 128 - s)
            pieces.append((0, s, p, n))
        else:
            n = min(P - p, NUM_SLOTS - s)
            pieces.append((1, s - 128, p, n))
        p += n
    return pieces


HEAD = 8  # layout-0 scatters issued before the rotated layouts are built


def _layout_seq(n_instr):
    seq = [0] * min(HEAD, n_instr)
    rr = (1, 2, 3, 0)
    while len(seq) < n_instr:
        seq.append(rr[(len(seq) - HEAD) % L])
    return seq


def _build_nc(n_instr):
    # Bacc: finalize() runs insert_act_table_loads (accurate Exp LUT) and
    # sync-wait legalization.
    nc = bacc.Bacc(None, num_swdge_queues=2)
    tbl_ext = nc.dram_tensor("table", [PAD_SLOTS, NUM_CATS], f32, kind="ExternalInput")
    offs_ext = nc.dram_tensor("offs", [P, n_instr], i32, kind="ExternalInput")
    out_ext = nc.dram_tensor("out", [TOK, NUM_CATS], f32, kind="ExternalOutput")
    flush_dram = nc.dram_tensor("flush", [P, 4], f32)

    probs = [nc.alloc_sbuf_tensor(f"probs{i}", [P, NUM_CATS], f32) for i in range(2)]
    expd = [nc.alloc_sbuf_tensor(f"expd{i}", [P, NUM_CATS], f32) for i in range(2)]
    sumexp = [nc.alloc_sbuf_tensor(f"sumexp{i}", [P, 1], f32) for i in range(2)]
    rcp = [nc.alloc_sbuf_tensor(f"rcp{i}", [P, 1], f32) for i in range(2)]
    offs_sb = nc.alloc_sbuf_tensor("offs_sb", [P, n_instr], i32)
    # rotated layouts 1..L-1 (layout 0 is probs0 itself)
    bigtbl = nc.alloc_sbuf_tensor("bigtbl", [P, (L - 1) * NUM_CATS], f32)

    n_pieces = sum(len(_rotation_pieces(b)) for b in ROTS[1:])

    def layout_ap(j):
        if j == 0:
            return probs[0].ap()[:]
        return bigtbl.ap()[:, (j - 1) * NUM_CATS:j * NUM_CATS]

    with (
        nc.Block() as block,
        nc.semaphore("s_load") as s_load,
        nc.semaphore("s_exp") as s_exp,
        nc.semaphore("s_prob") as s_prob,
        nc.semaphore("s_lay") as s_lay,
        nc.semaphore("s_sc") as s_sc,
        nc.semaphore("s_done") as s_done,
    ):

        @block.sync
        def _(sp: bass.BassEngine):
            for i in range(2):
                sp.dma_start(
                    out=probs[i].ap(), in_=tbl_ext[i * P:(i + 1) * P, :]
                ).then_inc(s_load, 16)
            sp.dma_start(out=offs_sb.ap(), in_=offs_ext[:]).then_inc(s_load, 16)
            # build rotated layout 1 once softmax finished (layouts 2-3 are
            # issued by the scalar engine in parallel)
            sp.wait_ge(s_prob, 2)
            for (chunk, src_lo, dst_lo, n) in _rotation_pieces(ROTS[1]):
                sp.dma_start(
                    out=bigtbl.ap()[dst_lo:dst_lo + n, 0:NUM_CATS],
                    in_=probs[chunk].ap()[src_lo:src_lo + n, :],
                ).then_inc(s_lay, 16)

        @block.vector
        def _(v: bass.BassEngine):
            # softmax without max-subtraction: inputs are N(0,1) (|x| < ~6),
            # exp is safe in fp32 and softmax is shift-invariant.
            v.wait_ge(s_exp, 2)
            for i in range(2):
                v.reciprocal(rcp[i].ap(), sumexp[i].ap())
            # same-engine RAW (rcp written above, read below) needs an
            # explicit pipeline drain in raw bass.
            v.drain()
            for i in range(2):
                ins = v.tensor_tensor(
                    out=probs[i].ap(), in0=expd[i].ap(),
                    in1=rcp[i].ap().to_broadcast([P, NUM_CATS]),
                    op=mybir.AluOpType.mult,
                )
                ins.then_inc(s_prob, 1)

        @block.scalar
        def _(a: bass.BassEngine):
            a.wait_ge(s_load, 48)
            for i in range(2):
                ins = a.activation(
                    out=expd[i].ap(), in_=probs[i].ap(),
                    func=mybir.ActivationFunctionType.Exp,
                    accum_out=sumexp[i].ap(),
                )
                ins.then_inc(s_exp, 1)
            a.wait_ge(s_prob, 2)
            for j, b in enumerate(ROTS[2:], start=1):
                for (chunk, src_lo, dst_lo, n) in _rotation_pieces(b):
                    a.dma_start(
                        out=bigtbl.ap()[dst_lo:dst_lo + n,
                                        j * NUM_CATS:(j + 1) * NUM_CATS],
                        in_=probs[chunk].ap()[src_lo:src_lo + n, :],
                    ).then_inc(s_lay, 16)

        seq = _layout_seq(n_instr)

        @block.gpsimd
        def _(g: bass.BassEngine):
            # head: layout-0 scatters only need probs0's softmax (first
            # s_prob increment); the rotated layouts gate the rest.
            g.wait_ge(s_prob, 1)
            breg = g.to_reg(TOK - 1)
            for i in range(n_instr):
                if i == HEAD:
                    g.wait_ge(s_lay, 16 * n_pieces)
                # walrus requires sync info on every DGE op; the exact count
                # is never waited on (the flush DMA is the completion
                # guarantee).
                ins = g.indirect_dma_start(
                    out=out_ext[:],
                    out_offset=bass.IndirectOffsetOnAxis(
                        ap=offs_sb.ap()[:, i:i + 1], axis=0
                    ),
                    in_=layout_ap(seq[i]),
                    in_offset=None,
                    bounds_check=breg,
                    oob_is_err=False,
                )
                ins.then_inc(s_sc, 16)
                if i % 2 == 1:
                    ins.ins.queue = "qPoolDynamic1"
            # flush: SWDGE per-engine rings drain in order, so when this
            # 128-partition marker lands, every scatter above has landed.
            g.dma_start(out=flush_dram[:], in_=probs[0].ap()[:, 0:4]).then_inc(
                s_done, 16
            )
            f2 = g.dma_start(out=flush_dram[:], in_=probs[0].ap()[:, 0:4])
            f2.then_inc(s_done, 16)
            f2.ins.queue = "qPoolDynamic1"
            g.wait_ge(s_done, 32)

    nc.finalize()
    return nc


_NC_CACHE = {}


def _get_nc(n_instr):
    if n_instr not in _NC_CACHE:
        _NC_CACHE[n_instr] = _build_nc(n_instr)
    return _NC_CACHE[n_instr]


def _pack_n(idx_c, n_instr):
    """Instruction i uses layout ROTS[i%L]; slot s is servable by the lane
    (s - b) % 168 when that value is < 128.  Spread each slot's tokens
    EVENLY over its serving instructions so every instruction keeps a
    similar lane count (keeps the scatter drain-bound end to end instead
    of a dense head and an emission-bound sparse tail)."""
    counts = np.bincount(idx_c, minlength=NUM_SLOTS)
    order = np.argsort(idx_c, kind="stable").astype(np.int64)
    starts = np.concatenate([[0], np.cumsum(counts)[:-1]])
    offs = np.full((P, n_instr), OOB, dtype=np.int32)
    seq = _layout_seq(n_instr)
    for s in range(NUM_SLOTS):
        n_s = counts[s]
        if n_s == 0:
            continue
        lanes = np.array([(s - ROTS[seq[i]]) % NUM_SLOTS for i in range(n_instr)])
        serving = np.where(lanes < P)[0]
        if n_s > len(serving):
            return None  # infeasible at this n_instr
        sel = serving[np.linspace(0, len(serving) - 1, n_s).round().astype(np.int64)]
        offs[lanes[sel], sel] = order[starts[s]:starts[s] + n_s]
    return offs


def _min_feasible_n(idx_c):
    counts = np.bincount(idx_c, minlength=NUM_SLOTS)
    n = max(TOK // P, int(counts.max()))
    while _pack_n(idx_c, n) is None:
        n += 1
    return n


def _run(inputs, trace=False):
    ih = np.asarray(inputs["inputs_hour"])
    tb = np.asarray(inputs["catid_time_matrix"], dtype=np.float32)
    tb_pad = np.zeros((PAD_SLOTS, NUM_CATS), dtype=np.float32)
    tb_pad[:NUM_SLOTS] = tb
    idx_full = np.ascontiguousarray(ih.astype(np.int32).reshape(BATCH * SEQ))

    shards = [idx_full[c * TOK:(c + 1) * TOK] for c in range(N_CORES)]
    n_instr = max(_min_feasible_n(s) for s in shards)
    per_core = [_pack_n(s, n_instr) for s in shards]

    nc = _get_nc(n_instr)
    in_maps = [
        {"table": tb_pad, "offs": np.ascontiguousarray(per_core[c])}
        for c in range(N_CORES)
    ]
    res = run_bass_kernel_spmd(nc, in_maps, core_ids=list(range(N_CORES)), trace=trace)
    outs = [res.results[i]["out"].reshape(B_CORE, SEQ, NUM_CATS) for i in range(N_CORES)]
    full = np.concatenate(outs, axis=0)
    return full, res


def kernel(**inputs):
    full, _ = _run(inputs, trace=False)
    return full



# revision 12
# speedup vs baseline: 1.6242x; 1.6242x over previous
"""Trainium2 Bass kernel: softmax(catid_time_matrix) row-gather (embedding lookup).

reference:
    probs = softmax(catid_time_matrix, axis=1)   # [168, 2048] fp32
    out   = probs[inputs_hour]                   # [512, 200, 2048] fp32

Strategy (8 NeuronCores, data-parallel over batch, bf16 payload):
  - Each core handles 64 batches = 12800 tokens, written as indirect
    scatter-DMAs straight from SBUF to dynamic DRAM row offsets (HBM
    traffic is write-only).
  - Rows are written in bfloat16; the host upconverts with an exact
    u16<<16 shift.  Frobenius rel-err vs the fp32 reference is ~1.7e-3,
    far inside the 2e-2 gate, and it halves HBM writes: 50 MB/core.
  - The scatter primitive moves one row per partition per instruction
    (128 max) and costs ~1.45 us of gpsimd SWDGE ucode per instruction,
    so the kernel is issue-bound: minimize instruction count by filling
    all 128 lanes every time.  A static rotation layout can serve a slot
    on only one lane, which forces ~142 half-filled instructions (the
    old kernel); instead the table lives in SBUF as 11 copies:
      - blocks 0-3: static rotations (partition p holds slot (p+42j)%168),
        built by loading the raw fp32 table from DRAM 4x with rotated row
        ranges, then per-rotation exp (scalar engine, with row-sum
        accumulation) and 1/sum scaling (vector engine, bf16 output).
      - blocks 4-10: data-dependent HOT layouts with per-slot multiplicity
        proportional to the shard's slot histogram deficit.  The softmaxed
        bf16 table is first written back to DRAM (0.7 MB), then one
        dma_gather instruction (896 int16 row indices, an input) builds
        all 7 blocks in one shot.
    Hot-slot duplication lets ~105 instructions carry all 12800 rows
    (95% lane fill) instead of 142.
  - The host packs token positions into (instruction, lane) cells; every
    cell serves exactly one slot, so assignment is conflict-free.  Unused
    lanes carry an out-of-bounds sentinel which the DMA bounds-check
    skips.  Never wait on exact scatter-semaphore counts (OOB-skipped
    engine stripes can under-increment); the trailing flush DMA on the
    same SWDGE queue is the completion guarantee (per-engine rings drain
    in order).
"""

import numpy as np

import concourse.bass as bass
import concourse.mybir as mybir
from concourse import bacc
from concourse.bass_utils import run_bass_kernel_spmd

NUM_SLOTS = 168
NUM_CATS = 2048
BATCH, SEQ = 512, 200
N_CORES = 8
B_CORE = BATCH // N_CORES       # 64 batches per core
TOK = B_CORE * SEQ              # 12800 tokens per core
P = 128
ROTS = (0, 42, 84, 126)         # block j<4: partition p holds slot (p+42j)%168
G = 7                           # gathered hot blocks 4..10
L = 4 + G
GATHER_AT = 8                   # gather issued after this many scatters
G_START = 18                    # first instruction allowed to use hot blocks
NIDX = G * P                    # gathered rows
OOB = np.int32(2**31 - 2)       # > bounds_check -> row silently skipped

f32 = mybir.dt.float32
bf16 = mybir.dt.bfloat16
i32 = mybir.dt.int32
i16 = mybir.dt.int16

# LANE[s][j]: partition serving slot s in rotation block j (or -1)
LANE = np.full((NUM_SLOTS, 4), -1, dtype=np.int64)
for _s in range(NUM_SLOTS):
    for _j, _b in enumerate(ROTS):
        _d = (_s - _b) % NUM_SLOTS
        if _d < P:
            LANE[_s, _j] = _d


def _seq_of(n_instr):
    return [i % 4 if i < G_START else i % L for i in range(n_instr)]


def _rotation_pieces(b):
    """Contiguous (src_row, dst_partition, n) ranges loading table rows
    (p+b)%168 onto partitions p."""
    pieces = []
    p = 0
    while p < P:
        s = (p + b) % NUM_SLOTS
        n = min(P - p, NUM_SLOTS - s)
        pieces.append((s, p, n))
        p += n
    return pieces


PIECES = [_rotation_pieces(b) for b in ROTS]


def _build_nc(n_instr):
    nc = bacc.Bacc(None)
    tbl_ext = nc.dram_tensor("table", [NUM_SLOTS, NUM_CATS], f32, kind="ExternalInput")
    offs_ext = nc.dram_tensor("offs", [P, n_instr], i32, kind="ExternalInput")
    # 16-partition wrapped index pattern, replicated on all 128 partitions
    # (each DMA-engine cluster reads its own copy — see swdge_reclaim_perf).
    gidx_ext = nc.dram_tensor("gidx", [P, NIDX // 16], i16, kind="ExternalInput")
    out_ext = nc.dram_tensor("out", [TOK, NUM_CATS], bf16, kind="ExternalOutput")
    probs_d = nc.dram_tensor("probs_d", [NUM_SLOTS, NUM_CATS], bf16)
    flush_dram = nc.dram_tensor("flush", [P, 4], bf16)

    rot = nc.alloc_sbuf_tensor("rot", [P, 4 * NUM_CATS], f32)
    expd = nc.alloc_sbuf_tensor("expd", [P, 4 * NUM_CATS], f32)
    sumexp = nc.alloc_sbuf_tensor("sumexp", [P, 4], f32)
    rcp = nc.alloc_sbuf_tensor("rcp", [P, 4], f32)
    lay = nc.alloc_sbuf_tensor("lay", [P, L * NUM_CATS], bf16)
    offs_sb = nc.alloc_sbuf_tensor("offs_sb", [P, n_instr], i32)
    gidx_sb = nc.alloc_sbuf_tensor("gidx_sb", [P, NIDX // 16], i16)

    def cslice(t, j):
        return t.ap()[:, j * NUM_CATS:(j + 1) * NUM_CATS]

    seq = _seq_of(n_instr)

    with (
        nc.Block() as block,
        nc.semaphore("s_r0") as s_r0,
        nc.semaphore("s_r1") as s_r1,
        nc.semaphore("s_r2") as s_r2,
        nc.semaphore("s_r3") as s_r3,
        nc.semaphore("s_off") as s_off,
        nc.semaphore("s_exp") as s_exp,
        nc.semaphore("s_lay") as s_lay,
        nc.semaphore("s_pd") as s_pd,
        nc.semaphore("s_g") as s_g,
        nc.semaphore("s_sc") as s_sc,
        nc.semaphore("s_done") as s_done,
    ):
        s_r = (s_r0, s_r1, s_r2, s_r3)

        @block.sync
        def _(sp: bass.BassEngine):
            # rotations 0,1 on the sync engine's HWDGE queue (2,3 plus the
            # offset/gather-index loads ride the scalar engine's queue)
            for j in (0, 1):
                for (src, dst, n) in PIECES[j]:
                    sp.dma_start(
                        out=rot.ap()[dst:dst + n, j * NUM_CATS:(j + 1) * NUM_CATS],
                        in_=tbl_ext[src:src + n, :],
                    ).then_inc(s_r[j], 16)
            # bf16 softmax table back to DRAM for the hot-block gather:
            # block 0 holds slots 0..127, block 3 (rot 126) holds slots
            # 128..167 on partitions 2..41.
            sp.wait_ge(s_lay, 1)
            sp.dma_start(out=probs_d[0:P, :], in_=cslice(lay, 0)).then_inc(s_pd, 16)
            sp.wait_ge(s_lay, 4)
            sp.dma_start(
                out=probs_d[P:NUM_SLOTS, :],
                in_=lay.ap()[2:2 + NUM_SLOTS - P, 3 * NUM_CATS:4 * NUM_CATS],
            ).then_inc(s_pd, 16)

        @block.scalar
        def _(a: bass.BassEngine):
            a.dma_start(out=offs_sb.ap(), in_=offs_ext[:]).then_inc(s_off, 16)
            a.dma_start(out=gidx_sb.ap(), in_=gidx_ext[:]).then_inc(s_off, 16)
            for j in (2, 3):
                for (src, dst, n) in PIECES[j]:
                    a.dma_start(
                        out=rot.ap()[dst:dst + n, j * NUM_CATS:(j + 1) * NUM_CATS],
                        in_=tbl_ext[src:src + n, :],
                    ).then_inc(s_r[j], 16)
            # softmax without max-subtraction: inputs are N(0,1), exp is safe
            # in fp32 and softmax is shift-invariant.
            for j in range(4):
                a.wait_ge(s_r[j], 16 * len(PIECES[j]))
                a.activation(
                    out=cslice(expd, j), in_=cslice(rot, j),
                    func=mybir.ActivationFunctionType.Exp,
                    accum_out=sumexp.ap()[:, j:j + 1],
                ).then_inc(s_exp, 1)

        @block.vector
        def _(v: bass.BassEngine):
            for j in range(4):
                v.wait_ge(s_exp, j + 1)
                v.reciprocal(rcp.ap()[:, j:j + 1], sumexp.ap()[:, j:j + 1])
                # same-engine RAW (rcp written above, read below) needs an
                # explicit pipeline drain in raw bass.
                v.drain()
                v.tensor_tensor(
                    out=cslice(lay, j), in0=cslice(expd, j),
                    in1=rcp.ap()[:, j:j + 1].to_broadcast([P, NUM_CATS]),
                    op=mybir.AluOpType.mult,
                ).then_inc(s_lay, 1)

        @block.gpsimd
        def _(g: bass.BassEngine):
            g.wait_ge(s_off, 32)
            breg = g.to_reg(TOK - 1)
            for i in range(n_instr):
                if i < 4:
                    g.wait_ge(s_lay, i + 1)
                if i == GATHER_AT:
                    g.wait_ge(s_pd, 32)
                    g.dma_gather(
                        out_ap=lay.ap()[:, 4 * NUM_CATS:L * NUM_CATS].rearrange(
                            "p (g c) -> p g c", c=NUM_CATS
                        ),
                        in_ap=probs_d[:],
                        idxs_ap=gidx_sb.ap(),
                        num_idxs=NIDX,
                        num_idxs_reg=NIDX,
                        elem_size=NUM_CATS,
                    ).then_inc(s_g, 16)
                if i == G_START:
                    g.wait_ge(s_g, 16)
                g.indirect_dma_start(
                    out=out_ext[:],
                    out_offset=bass.IndirectOffsetOnAxis(
                        ap=offs_sb.ap()[:, i:i + 1], axis=0
                    ),
                    in_=cslice(lay, seq[i]),
                    in_offset=None,
                    bounds_check=breg,
                    oob_is_err=False,
                ).then_inc(s_sc, 16)  # sync info required; count never waited on
            # flush: SWDGE per-engine rings drain in order, so once this
            # 128-partition marker lands, every scatter above has landed.
            g.dma_start(out=flush_dram[:], in_=lay.ap()[:, 0:4]).then_inc(
                s_done, 16
            )
            g.wait_ge(s_done, 16)

    nc.finalize()
    return nc


_NC_CACHE = {}


def _get_nc(n_instr):
    if n_instr not in _NC_CACHE:
        _NC_CACHE[n_instr] = _build_nc(n_instr)
    return _NC_CACHE[n_instr]


def _design_core(idx_c, n_instr):
    """Design hot blocks + pack tokens.  Returns (offs [P,n] i32,
    gidx [16, NIDX//16] i16) or None if infeasible at this n_instr."""
    counts = np.bincount(idx_c, minlength=NUM_SLOTS)
    seq = _seq_of(n_instr)
    use = np.bincount(seq, minlength=L)

    cap_rot = np.zeros(NUM_SLOTS)
    for s in range(NUM_SLOTS):
        for j in range(4):
            if LANE[s, j] >= 0:
                cap_rot[s] += use[j]

    # hot lanes: give the next copy to the slot with the largest remaining
    # deficit, placing into the emptiest block
    hot_blocks = [[] for _ in range(G)]
    rem = np.maximum(0, counts - cap_rot).astype(np.float64)
    u_hot = use[4:].astype(np.float64)
    lanes_used = 0
    while lanes_used < G * P and rem.max() > 0:
        s = int(np.argmax(rem))
        sizes = [len(h) for h in hot_blocks]
        gi = int(np.argmin(sizes))
        if sizes[gi] >= P:
            break
        hot_blocks[gi].append(s)
        rem[s] -= u_hot[gi]
        lanes_used += 1
    if rem.max() > 0:
        return None
    hottest = int(np.argmax(counts))
    for gi in range(G):
        while len(hot_blocks[gi]) < P:
            hot_blocks[gi].append(hottest)

    # lane lookup for hot blocks: block g, slot s -> lanes
    lane_of_hot = [dict() for _ in range(G)]
    for gi in range(G):
        for p, s in enumerate(hot_blocks[gi]):
            lane_of_hot[gi].setdefault(s, []).append(p)

    order = np.argsort(idx_c, kind="stable").astype(np.int64)
    starts = np.concatenate([[0], np.cumsum(counts)[:-1]])
    offs = np.full((P, n_instr), OOB, dtype=np.int32)
    for s in range(NUM_SLOTS):
        n_s = counts[s]
        if n_s == 0:
            continue
        cells = []
        for i in range(n_instr):
            b = seq[i]
            if b < 4:
                if LANE[s, b] >= 0:
                    cells.append((i, int(LANE[s, b])))
            else:
                for p in lane_of_hot[b - 4].get(s, []):
                    cells.append((i, p))
        if n_s > len(cells):
            return None
        sel = np.linspace(0, len(cells) - 1, n_s).round().astype(np.int64)
        toks = order[starts[s]:starts[s] + n_s]
        for t, ci in zip(toks, sel):
            i, p = cells[ci]
            offs[p, i] = t

    gflat = np.concatenate([np.asarray(hot_blocks[gi], np.int16) for gi in range(G)])
    gidx16 = np.zeros((16, NIDX // 16), dtype=np.int16)
    for i, s in enumerate(gflat):
        gidx16[i % 16, i // 16] = s
    gidx = np.tile(gidx16, (P // 16, 1))
    return offs, gidx


def _min_feasible_n(idx_c):
    n = max(G_START + 1, (TOK + P - 1) // P)
    while _design_core(idx_c, n) is None:
        n += 1
    return n


def _bf16_to_f32(a):
    return (a.view(np.uint16).astype(np.uint32) << np.uint32(16)).view(np.float32)


def _run(inputs, trace=False):
    ih = np.asarray(inputs["inputs_hour"])
    tb = np.ascontiguousarray(np.asarray(inputs["catid_time_matrix"], dtype=np.float32))
    idx_full = np.ascontiguousarray(ih.astype(np.int32).reshape(BATCH * SEQ))

    shards = [idx_full[c * TOK:(c + 1) * TOK] for c in range(N_CORES)]
    n_instr = max(_min_feasible_n(s) for s in shards)
    designs = [_design_core(s, n_instr) for s in shards]

    nc = _get_nc(n_instr)
    in_maps = [
        {
            "table": tb,
            "offs": np.ascontiguousarray(designs[c][0]),
            "gidx": np.ascontiguousarray(designs[c][1]),
        }
        for c in range(N_CORES)
    ]
    res = run_bass_kernel_spmd(nc, in_maps, core_ids=list(range(N_CORES)), trace=trace)
    outs = [
        _bf16_to_f32(np.asarray(res.results[i]["out"])).reshape(B_CORE, SEQ, NUM_CATS)
        for i in range(N_CORES)
    ]
    full = np.concatenate(outs, axis=0)
    return full, res


def kernel(**inputs):
    full, _ = _run(inputs, trace=False)
    return full


# revision 19
# speedup vs baseline: 1.6885x; 1.0396x over previous
"""Trainium2 Bass kernel: softmax(catid_time_matrix) row-gather (embedding lookup).

reference:
    probs = softmax(catid_time_matrix, axis=1)   # [168, 2048] fp32
    out   = probs[inputs_hour]                   # [512, 200, 2048] fp32

Strategy (8 NeuronCores, data-parallel over batch, bf16 payload):
  - Each core handles 64 batches = 12800 tokens, written as indirect
    scatter-DMAs straight from SBUF to dynamic DRAM row offsets (HBM
    traffic is write-only).
  - Rows are written in bfloat16; the host upconverts with an exact
    u16<<16 shift.  Frobenius rel-err vs the fp32 reference is ~1.7e-3,
    far inside the 2e-2 gate, and it halves HBM writes: 50 MB/core.
  - The scatter primitive moves one row per partition per instruction
    (128 max) and costs ~1.45 us of gpsimd SWDGE ucode per instruction,
    so the kernel is issue-bound: minimize instruction count by filling
    all 128 lanes every time.  A static rotation layout can serve a slot
    on only one lane, which forces ~142 half-filled instructions (the
    old kernel); instead the table lives in SBUF as 11 copies:
      - blocks 0-3: static rotations (partition p holds slot (p+42j)%168),
        built by loading the raw fp32 table from DRAM 4x with rotated row
        ranges, then per-rotation exp (scalar engine, with row-sum
        accumulation) and 1/sum scaling (vector engine, bf16 output).
      - blocks 4-10: data-dependent HOT layouts with per-slot multiplicity
        proportional to the shard's slot histogram deficit.  The softmaxed
        bf16 table is first written back to DRAM (0.7 MB), then one
        dma_gather instruction (896 int16 row indices, an input) builds
        all 7 blocks in one shot.
    Hot-slot duplication lets ~105 instructions carry all 12800 rows
    (95% lane fill) instead of 142.
  - The host packs token positions into (instruction, lane) cells; every
    cell serves exactly one slot, so assignment is conflict-free.  Unused
    lanes carry an out-of-bounds sentinel which the DMA bounds-check
    skips.  Never wait on exact scatter-semaphore counts (OOB-skipped
    engine stripes can under-increment); the trailing flush DMA on the
    same SWDGE queue is the completion guarantee (per-engine rings drain
    in order).
"""

import numpy as np

import concourse.bass as bass
import concourse.mybir as mybir
from concourse import bacc
from concourse.bass_utils import run_bass_kernel_spmd

NUM_SLOTS = 168
NUM_CATS = 2048
BATCH, SEQ = 512, 200
N_CORES = 8
B_CORE = BATCH // N_CORES       # 64 batches per core
TOK = B_CORE * SEQ              # 12800 tokens per core
P = 128
ROTS = (0, 42, 84, 126)         # block j<4: partition p holds slot (p+42j)%168
G = 7                           # gathered hot blocks 4..10
L = 4 + G
GATHER_AT = 8                   # gather issued after this many scatters
G_START = 18                    # first instruction allowed to use hot blocks
NIDX = G * P                    # gathered rows
OOB = np.int32(2**31 - 2)       # > bounds_check -> row silently skipped

f32 = mybir.dt.float32
bf16 = mybir.dt.bfloat16
i32 = mybir.dt.int32
i16 = mybir.dt.int16

# LANE[s][j]: partition serving slot s in rotation block j (or -1)
LANE = np.full((NUM_SLOTS, 4), -1, dtype=np.int64)
for _s in range(NUM_SLOTS):
    for _j, _b in enumerate(ROTS):
        _d = (_s - _b) % NUM_SLOTS
        if _d < P:
            LANE[_s, _j] = _d


HEAD_SEQ = (0, 0, 1, 1, 2, 2, 3, 3)  # matches rotation-readiness order


def _seq_of(n_instr):
    seq = []
    for i in range(n_instr):
        if i < len(HEAD_SEQ):
            seq.append(HEAD_SEQ[i])
        elif i < G_START:
            seq.append(i % 4)
        else:
            seq.append(i % L)
    return seq


def _rotation_pieces(b):
    """Contiguous (src_row, dst_partition, n) ranges loading table rows
    (p+b)%168 onto partitions p."""
    pieces = []
    p = 0
    while p < P:
        s = (p + b) % NUM_SLOTS
        n = min(P - p, NUM_SLOTS - s)
        pieces.append((s, p, n))
        p += n
    return pieces


PIECES = [_rotation_pieces(b) for b in ROTS]


def _build_nc(n_instr):
    nc = bacc.Bacc(None)
    tbl_ext = nc.dram_tensor("table", [NUM_SLOTS, NUM_CATS], f32, kind="ExternalInput")
    offs_ext = nc.dram_tensor("offs", [P, n_instr], i32, kind="ExternalInput")
    # 16-partition wrapped index pattern, replicated on all 128 partitions
    # (each DMA-engine cluster reads its own copy — see swdge_reclaim_perf).
    gidx_ext = nc.dram_tensor("gidx", [P, NIDX // 16], i16, kind="ExternalInput")
    out_ext = nc.dram_tensor("out", [TOK, NUM_CATS], bf16, kind="ExternalOutput")
    probs_d = nc.dram_tensor("probs_d", [NUM_SLOTS, NUM_CATS], bf16)
    flush_dram = nc.dram_tensor("flush", [P, 4], bf16)

    rot = nc.alloc_sbuf_tensor("rot", [P, 4 * NUM_CATS], f32)
    expd = nc.alloc_sbuf_tensor("expd", [P, 4 * NUM_CATS], bf16)
    sumexp = nc.alloc_sbuf_tensor("sumexp", [P, 4], f32)
    rcp = nc.alloc_sbuf_tensor("rcp", [P, 4], bf16)
    lay = nc.alloc_sbuf_tensor("lay", [P, L * NUM_CATS], bf16)
    offs_sb = nc.alloc_sbuf_tensor("offs_sb", [P, n_instr], i32)
    gidx_sb = nc.alloc_sbuf_tensor("gidx_sb", [P, NIDX // 16], i16)

    def cslice(t, j):
        return t.ap()[:, j * NUM_CATS:(j + 1) * NUM_CATS]

    seq = _seq_of(n_instr)

    with (
        nc.Block() as block,
        nc.semaphore("s_r0") as s_r0,
        nc.semaphore("s_r1") as s_r1,
        nc.semaphore("s_r2") as s_r2,
        nc.semaphore("s_r3") as s_r3,
        nc.semaphore("s_off") as s_off,
        nc.semaphore("s_exp") as s_exp,
        nc.semaphore("s_lay") as s_lay,
        nc.semaphore("s_pd") as s_pd,
        nc.semaphore("s_g") as s_g,
        nc.semaphore("s_sc") as s_sc,
        nc.semaphore("s_done") as s_done,
    ):
        s_r = (s_r0, s_r1, s_r2, s_r3)

        @block.sync
        def _(sp: bass.BassEngine):
            # All loads ride the sync engine's HWDGE queue (the scalar
            # engine's queue measured ~10x slower) except rotations 2,3
            # which the gpsimd engine issues on its own SWDGE queue.
            sp.dma_start(out=offs_sb.ap(), in_=offs_ext[:]).then_inc(s_off, 16)
            sp.dma_start(out=gidx_sb.ap(), in_=gidx_ext[:]).then_inc(s_off, 16)
            for j in (0, 1):
                for (src, dst, n) in PIECES[j]:
                    sp.dma_start(
                        out=rot.ap()[dst:dst + n, j * NUM_CATS:(j + 1) * NUM_CATS],
                        in_=tbl_ext[src:src + n, :],
                    ).then_inc(s_r[j], 16)
            # bf16 softmax table back to DRAM for the hot-block gather:
            # block 0 holds slots 0..127, block 3 (rot 126) holds slots
            # 128..167 on partitions 2..41.
            sp.wait_ge(s_lay, 1)
            sp.dma_start(out=probs_d[0:P, :], in_=cslice(lay, 0)).then_inc(s_pd, 16)
            sp.wait_ge(s_lay, 4)
            sp.dma_start(
                out=probs_d[P:NUM_SLOTS, :],
                in_=lay.ap()[2:2 + NUM_SLOTS - P, 3 * NUM_CATS:4 * NUM_CATS],
            ).then_inc(s_pd, 16)

        @block.scalar
        def _(a: bass.BassEngine):
            # softmax without max-subtraction: inputs are N(0,1), exp is safe
            # in fp32 and softmax is shift-invariant.
            for j in range(4):
                a.wait_ge(s_r[j], 16 * len(PIECES[j]))
                a.activation(
                    out=cslice(expd, j), in_=cslice(rot, j),
                    func=mybir.ActivationFunctionType.Exp,
                    accum_out=sumexp.ap()[:, j:j + 1],
                ).then_inc(s_exp, 1)

        @block.vector
        def _(v: bass.BassEngine):
            with nc.allow_low_precision(reason="bf16 payload; 2e-2 gate"):
                for j in range(4):
                    v.wait_ge(s_exp, j + 1)
                    v.reciprocal(rcp.ap()[:, j:j + 1], sumexp.ap()[:, j:j + 1])
                    # same-engine RAW (rcp written above, read below) needs
                    # an explicit pipeline drain in raw bass.
                    v.drain()
                    v.tensor_tensor(
                        out=cslice(lay, j), in0=cslice(expd, j),
                        in1=rcp.ap()[:, j:j + 1].to_broadcast([P, NUM_CATS]),
                        op=mybir.AluOpType.mult,
                    ).then_inc(s_lay, 1)

        @block.gpsimd
        def _(g: bass.BassEngine):
            breg = g.to_reg(TOK - 1)
            for j in (2, 3):
                for (src, dst, n) in PIECES[j]:
                    g.dma_start(
                        out=rot.ap()[dst:dst + n, j * NUM_CATS:(j + 1) * NUM_CATS],
                        in_=tbl_ext[src:src + n, :],
                    ).then_inc(s_r[j], 16)
            g.wait_ge(s_off, 32)
            for i in range(n_instr):
                if i < 4:
                    g.wait_ge(s_lay, i + 1)
                if i == GATHER_AT:
                    g.wait_ge(s_pd, 32)
                    g.dma_gather(
                        out_ap=lay.ap()[:, 4 * NUM_CATS:L * NUM_CATS].rearrange(
                            "p (g c) -> p g c", c=NUM_CATS
                        ),
                        in_ap=probs_d[:],
                        idxs_ap=gidx_sb.ap(),
                        num_idxs=NIDX,
                        num_idxs_reg=NIDX,
                        elem_size=NUM_CATS,
                    ).then_inc(s_g, 16)
                if i == G_START:
                    g.wait_ge(s_g, 16)
                g.indirect_dma_start(
                    out=out_ext[:],
                    out_offset=bass.IndirectOffsetOnAxis(
                        ap=offs_sb.ap()[:, i:i + 1], axis=0
                    ),
                    in_=cslice(lay, seq[i]),
                    in_offset=None,
                    bounds_check=breg,
                    oob_is_err=False,
                ).then_inc(s_sc, 16)  # sync info required; count never waited on
            # flush: SWDGE per-engine rings drain in order, so once this
            # 128-partition marker lands, every scatter above has landed.
            g.dma_start(out=flush_dram[:], in_=lay.ap()[:, 0:4]).then_inc(
                s_done, 16
            )
            g.wait_ge(s_done, 16)

    nc.finalize()
    return nc


_NC_CACHE = {}


def _get_nc(n_instr):
    if n_instr not in _NC_CACHE:
        _NC_CACHE[n_instr] = _build_nc(n_instr)
    return _NC_CACHE[n_instr]


def _design_core(idx_c, n_instr):
    """Design hot blocks + pack tokens.  Returns (offs [P,n] i32,
    gidx [16, NIDX//16] i16) or None if infeasible at this n_instr."""
    counts = np.bincount(idx_c, minlength=NUM_SLOTS)
    seq = _seq_of(n_instr)
    use = np.bincount(seq, minlength=L)

    cap_rot = np.zeros(NUM_SLOTS)
    for s in range(NUM_SLOTS):
        for j in range(4):
            if LANE[s, j] >= 0:
                cap_rot[s] += use[j]

    # hot lanes: give the next copy to the slot with the largest remaining
    # deficit, placing into the emptiest block
    hot_blocks = [[] for _ in range(G)]
    rem = np.maximum(0, counts - cap_rot).astype(np.float64)
    u_hot = use[4:].astype(np.float64)
    lanes_used = 0
    while lanes_used < G * P and rem.max() > 0:
        s = int(np.argmax(rem))
        sizes = [len(h) for h in hot_blocks]
        gi = int(np.argmin(sizes))
        if sizes[gi] >= P:
            break
        hot_blocks[gi].append(s)
        rem[s] -= u_hot[gi]
        lanes_used += 1
    if rem.max() > 0:
        return None
    hottest = int(np.argmax(counts))
    for gi in range(G):
        while len(hot_blocks[gi]) < P:
            hot_blocks[gi].append(hottest)

    # lane lookup for hot blocks: block g, slot s -> lanes
    lane_of_hot = [dict() for _ in range(G)]
    for gi in range(G):
        for p, s in enumerate(hot_blocks[gi]):
            lane_of_hot[gi].setdefault(s, []).append(p)

    order = np.argsort(idx_c, kind="stable").astype(np.int64)
    starts = np.concatenate([[0], np.cumsum(counts)[:-1]])
    offs = np.full((P, n_instr), OOB, dtype=np.int32)
    for s in range(NUM_SLOTS):
        n_s = counts[s]
        if n_s == 0:
            continue
        cells = []
        for i in range(n_instr):
            b = seq[i]
            if b < 4:
                if LANE[s, b] >= 0:
                    cells.append((i, int(LANE[s, b])))
            else:
                for p in lane_of_hot[b - 4].get(s, []):
                    cells.append((i, p))
        if n_s > len(cells):
            return None
        sel = np.linspace(0, len(cells) - 1, n_s).round().astype(np.int64)
        toks = order[starts[s]:starts[s] + n_s]
        for t, ci in zip(toks, sel):
            i, p = cells[ci]
            offs[p, i] = t

    gflat = np.concatenate([np.asarray(hot_blocks[gi], np.int16) for gi in range(G)])
    gidx16 = np.zeros((16, NIDX // 16), dtype=np.int16)
    for i, s in enumerate(gflat):
        gidx16[i % 16, i // 16] = s
    gidx = np.tile(gidx16, (P // 16, 1))
    return offs, gidx


def _min_feasible_n(idx_c):
    n = max(G_START + 1, (TOK + P - 1) // P)
    while _design_core(idx_c, n) is None:
        n += 1
    return n


def _bf16_to_f32(a):
    return (a.view(np.uint16).astype(np.uint32) << np.uint32(16)).view(np.float32)


def _run(inputs, trace=False):
    ih = np.asarray(inputs["inputs_hour"])
    tb = np.ascontiguousarray(np.asarray(inputs["catid_time_matrix"], dtype=np.float32))
    idx_full = np.ascontiguousarray(ih.astype(np.int32).reshape(BATCH * SEQ))

    shards = [idx_full[c * TOK:(c + 1) * TOK] for c in range(N_CORES)]
    n_instr = max(_min_feasible_n(s) for s in shards)
    designs = [_design_core(s, n_instr) for s in shards]

    nc = _get_nc(n_instr)
    in_maps = [
        {
            "table": tb,
            "offs": np.ascontiguousarray(designs[c][0]),
            "gidx": np.ascontiguousarray(designs[c][1]),
        }
        for c in range(N_CORES)
    ]
    res = run_bass_kernel_spmd(nc, in_maps, core_ids=list(range(N_CORES)), trace=trace)
    outs = [
        _bf16_to_f32(np.asarray(res.results[i]["out"])).reshape(B_CORE, SEQ, NUM_CATS)
        for i in range(N_CORES)
    ]
    full = np.concatenate(outs, axis=0)
    return full, res


def kernel(**inputs):
    full, _ = _run(inputs, trace=False)
    return full


# revision 23
# speedup vs baseline: 1.8557x; 1.0991x over previous
"""Trainium2 Bass kernel: softmax(catid_time_matrix) row-gather (embedding lookup).

reference:
    probs = softmax(catid_time_matrix, axis=1)   # [168, 2048] fp32
    out   = probs[inputs_hour]                   # [512, 200, 2048] fp32

Strategy (8 NeuronCores, data-parallel over batch, bf16 payload):
  - Each core handles 64 batches = 12800 tokens, written as indirect
    scatter-DMAs straight from SBUF to dynamic DRAM row offsets (HBM
    traffic is write-only).
  - Rows are written in bfloat16; the host upconverts with an exact
    u16<<16 shift.  Frobenius rel-err vs the fp32 reference is ~1.7e-3,
    far inside the 2e-2 gate, and it halves HBM writes: 50 MB/core.
  - The scatter primitive moves one row per partition per instruction
    (128 max) and costs ~1.45 us of gpsimd SWDGE ucode per instruction,
    so the kernel is issue-bound: minimize instruction count by filling
    all 128 lanes every time.  A static rotation layout can serve a slot
    on only one lane, which forces ~142 half-filled instructions (the
    old kernel); instead the table lives in SBUF as 11 copies:
      - blocks 0-3: static rotations (partition p holds slot (p+42j)%168),
        built by loading the raw fp32 table from DRAM 4x with rotated row
        ranges, then per-rotation exp (scalar engine, with row-sum
        accumulation) and 1/sum scaling (vector engine, bf16 output).
      - blocks 4-10: data-dependent HOT layouts with per-slot multiplicity
        proportional to the shard's slot histogram deficit.  The softmaxed
        bf16 table is first written back to DRAM (0.7 MB), then one
        dma_gather instruction (896 int16 row indices, an input) builds
        all 7 blocks in one shot.
    Hot-slot duplication lets ~105 instructions carry all 12800 rows
    (95% lane fill) instead of 142.
  - The host packs token positions into (instruction, lane) cells; every
    cell serves exactly one slot, so assignment is conflict-free.  Unused
    lanes carry an out-of-bounds sentinel which the DMA bounds-check
    skips.  Never wait on exact scatter-semaphore counts (OOB-skipped
    engine stripes can under-increment); the trailing flush DMA on the
    same SWDGE queue is the completion guarantee (per-engine rings drain
    in order).
"""

import numpy as np

import concourse.bass as bass
import concourse.mybir as mybir
from concourse import bacc
from concourse.bass_utils import run_bass_kernel_spmd

NUM_SLOTS = 168
NUM_CATS = 2048
BATCH, SEQ = 512, 200
N_CORES = 8
B_CORE = BATCH // N_CORES       # 64 batches per core
TOK = B_CORE * SEQ              # 12800 tokens per core
P = 128
ROTS = (0, 42, 84, 126)         # block j<4: partition p holds slot (p+42j)%168
G = 7                           # gathered hot blocks 4..10
L = 4 + G
GATHER_AT = 8                   # gather issued after this many scatters
G_START = 18                    # first instruction allowed to use hot blocks
NIDX = G * P                    # gathered rows
OOB = np.int32(2**31 - 2)       # > bounds_check -> row silently skipped

f32 = mybir.dt.float32
bf16 = mybir.dt.bfloat16
i32 = mybir.dt.int32
i16 = mybir.dt.int16

# LANE[s][j]: partition serving slot s in rotation block j (or -1)
LANE = np.full((NUM_SLOTS, 4), -1, dtype=np.int64)
for _s in range(NUM_SLOTS):
    for _j, _b in enumerate(ROTS):
        _d = (_s - _b) % NUM_SLOTS
        if _d < P:
            LANE[_s, _j] = _d


HEAD_SEQ = (0, 0, 1, 1, 2, 2, 3, 3)  # matches rotation-readiness order


def _seq_of(n_instr):
    seq = []
    for i in range(n_instr):
        if i < len(HEAD_SEQ):
            seq.append(HEAD_SEQ[i])
        elif i < G_START:
            seq.append(i % 4)
        else:
            seq.append(i % L)
    return seq


def _rotation_pieces(b):
    """Contiguous (src_row, dst_partition, n) ranges loading table rows
    (p+b)%168 onto partitions p."""
    pieces = []
    p = 0
    while p < P:
        s = (p + b) % NUM_SLOTS
        n = min(P - p, NUM_SLOTS - s)
        pieces.append((s, p, n))
        p += n
    return pieces


PIECES = [_rotation_pieces(b) for b in ROTS]


def _build_nc(n_instr):
    nc = bacc.Bacc(None)
    # table is uploaded pre-cast to bf16: rotation loads are 4 reads of the
    # full table and all 8 cores load concurrently, so halving the bytes
    # halves the critical-path load time.
    tbl_ext = nc.dram_tensor("table", [NUM_SLOTS, NUM_CATS], bf16, kind="ExternalInput")
    offs_ext = nc.dram_tensor("offs", [P, n_instr], i32, kind="ExternalInput")
    # 16-partition wrapped index pattern, replicated on all 128 partitions
    # (each DMA-engine cluster reads its own copy — see swdge_reclaim_perf).
    gidx_ext = nc.dram_tensor("gidx", [P, NIDX // 16], i16, kind="ExternalInput")
    out_ext = nc.dram_tensor("out", [TOK, NUM_CATS], bf16, kind="ExternalOutput")
    probs_d = nc.dram_tensor("probs_d", [NUM_SLOTS, NUM_CATS], bf16)
    flush_dram = nc.dram_tensor("flush", [P, 4], bf16)

    rot = nc.alloc_sbuf_tensor("rot", [P, 4 * NUM_CATS], bf16)
    gscr = nc.alloc_sbuf_tensor("gscr", [P, NUM_CATS], bf16)
    expd = nc.alloc_sbuf_tensor("expd", [P, 4 * NUM_CATS], bf16)
    sumexp = nc.alloc_sbuf_tensor("sumexp", [P, 4], f32)
    rcp = nc.alloc_sbuf_tensor("rcp", [P, 4], bf16)
    lay = nc.alloc_sbuf_tensor("lay", [P, L * NUM_CATS], bf16)
    offs_sb = nc.alloc_sbuf_tensor("offs_sb", [P, n_instr], i32)
    gidx_sb = nc.alloc_sbuf_tensor("gidx_sb", [P, NIDX // 16], i16)

    def cslice(t, j):
        return t.ap()[:, j * NUM_CATS:(j + 1) * NUM_CATS]

    seq = _seq_of(n_instr)

    with (
        nc.Block() as block,
        nc.semaphore("s_r0") as s_r0,
        nc.semaphore("s_r1") as s_r1,
        nc.semaphore("s_r2") as s_r2,
        nc.semaphore("s_r3") as s_r3,
        nc.semaphore("s_off") as s_off,
        nc.semaphore("s_exp") as s_exp,
        nc.semaphore("s_lay") as s_lay,
        nc.semaphore("s_pd") as s_pd,
        nc.semaphore("s_g") as s_g,
        nc.semaphore("s_sc") as s_sc,
        nc.semaphore("s_done") as s_done,
    ):
        s_r = (s_r0, s_r1, s_r2, s_r3)

        @block.sync
        def _(sp: bass.BassEngine):
            # All loads ride the sync engine's HWDGE queue (the scalar
            # engine's queue measured ~10x slower) except rotations 2,3
            # which the gpsimd engine issues on its own SWDGE queue.
            sp.dma_start(out=offs_sb.ap(), in_=offs_ext[:]).then_inc(s_off, 16)
            sp.dma_start(out=gidx_sb.ap(), in_=gidx_ext[:]).then_inc(s_off, 16)
            for j in (0, 1):
                for (src, dst, n) in PIECES[j]:
                    sp.dma_start(
                        out=rot.ap()[dst:dst + n, j * NUM_CATS:(j + 1) * NUM_CATS],
                        in_=tbl_ext[src:src + n, :],
                    ).then_inc(s_r[j], 16)
            # bf16 softmax table back to DRAM for the hot-block gather:
            # block 0 holds slots 0..127, block 3 (rot 126) holds slots
            # 128..167 on partitions 2..41.
            sp.wait_ge(s_lay, 1)
            sp.dma_start(out=probs_d[0:P, :], in_=cslice(lay, 0)).then_inc(s_pd, 16)
            sp.wait_ge(s_lay, 4)
            sp.dma_start(
                out=probs_d[P:NUM_SLOTS, :],
                in_=lay.ap()[2:2 + NUM_SLOTS - P, 3 * NUM_CATS:4 * NUM_CATS],
            ).then_inc(s_pd, 16)

        @block.scalar
        def _(a: bass.BassEngine):
            # softmax without max-subtraction: inputs are N(0,1), exp is safe
            # in fp32 and softmax is shift-invariant.
            for j in range(4):
                a.wait_ge(s_r[j], 16 * len(PIECES[j]))
                a.activation(
                    out=cslice(expd, j), in_=cslice(rot, j),
                    func=mybir.ActivationFunctionType.Exp,
                    accum_out=sumexp.ap()[:, j:j + 1],
                ).then_inc(s_exp, 1)

        @block.vector
        def _(v: bass.BassEngine):
            with nc.allow_low_precision(reason="bf16 payload; 2e-2 gate"):
                for j in range(4):
                    v.wait_ge(s_exp, j + 1)
                    v.reciprocal(rcp.ap()[:, j:j + 1], sumexp.ap()[:, j:j + 1])
                    # same-engine RAW (rcp written above, read below) needs
                    # an explicit pipeline drain in raw bass.
                    v.drain()
                    v.tensor_tensor(
                        out=cslice(lay, j), in0=cslice(expd, j),
                        in1=rcp.ap()[:, j:j + 1].to_broadcast([P, NUM_CATS]),
                        op=mybir.AluOpType.mult,
                    ).then_inc(s_lay, 1)

        @block.gpsimd
        def _(g: bass.BassEngine):
            breg = g.to_reg(TOK - 1)
            for j in (2, 3):
                for (src, dst, n) in PIECES[j]:
                    g.dma_start(
                        out=rot.ap()[dst:dst + n, j * NUM_CATS:(j + 1) * NUM_CATS],
                        in_=tbl_ext[src:src + n, :],
                    ).then_inc(s_r[j], 16)
            g.wait_ge(s_off, 32)
            # dummy 16-row gather: forces the q7 ucode library load (~9 us)
            # NOW, overlapped with the softmax chain, so the real gather
            # below doesn't pay it mid-scatter-stream.
            g.dma_gather(
                out_ap=gscr.ap().rearrange("p (g c) -> p g c", g=1),
                in_ap=tbl_ext[:],
                idxs_ap=gidx_sb.ap()[:, 0:1],
                num_idxs=16,
                num_idxs_reg=16,
                elem_size=NUM_CATS,
            ).then_inc(s_sc, 16)
            for i in range(n_instr):
                if i < 4:
                    g.wait_ge(s_lay, i + 1)
                if i == GATHER_AT:
                    g.wait_ge(s_pd, 32)
                    g.dma_gather(
                        out_ap=lay.ap()[:, 4 * NUM_CATS:L * NUM_CATS].rearrange(
                            "p (g c) -> p g c", c=NUM_CATS
                        ),
                        in_ap=probs_d[:],
                        idxs_ap=gidx_sb.ap(),
                        num_idxs=NIDX,
                        num_idxs_reg=NIDX,
                        elem_size=NUM_CATS,
                    ).then_inc(s_g, 16)
                if i == G_START:
                    g.wait_ge(s_g, 16)
                g.indirect_dma_start(
                    out=out_ext[:],
                    out_offset=bass.IndirectOffsetOnAxis(
                        ap=offs_sb.ap()[:, i:i + 1], axis=0
                    ),
                    in_=cslice(lay, seq[i]),
                    in_offset=None,
                    bounds_check=breg,
                    oob_is_err=False,
                ).then_inc(s_sc, 16)  # sync info required; count never waited on
            # flush: SWDGE per-engine rings drain in order, so once this
            # 128-partition marker lands, every scatter above has landed.
            g.dma_start(out=flush_dram[:], in_=lay.ap()[:, 0:4]).then_inc(
                s_done, 16
            )
            g.wait_ge(s_done, 16)

    nc.finalize()
    return nc


_NC_CACHE = {}


def _get_nc(n_instr):
    if n_instr not in _NC_CACHE:
        _NC_CACHE[n_instr] = _build_nc(n_instr)
    return _NC_CACHE[n_instr]


def _design_core(idx_c, n_instr):
    """Design hot blocks + pack tokens.  Returns (offs [P,n] i32,
    gidx [16, NIDX//16] i16) or None if infeasible at this n_instr."""
    counts = np.bincount(idx_c, minlength=NUM_SLOTS)
    seq = _seq_of(n_instr)
    use = np.bincount(seq, minlength=L)

    cap_rot = np.zeros(NUM_SLOTS)
    for s in range(NUM_SLOTS):
        for j in range(4):
            if LANE[s, j] >= 0:
                cap_rot[s] += use[j]

    # hot lanes: give the next copy to the slot with the largest remaining
    # deficit, placing into the emptiest block
    hot_blocks = [[] for _ in range(G)]
    rem = np.maximum(0, counts - cap_rot).astype(np.float64)
    u_hot = use[4:].astype(np.float64)
    lanes_used = 0
    while lanes_used < G * P and rem.max() > 0:
        s = int(np.argmax(rem))
        sizes = [len(h) for h in hot_blocks]
        gi = int(np.argmin(sizes))
        if sizes[gi] >= P:
            break
        hot_blocks[gi].append(s)
        rem[s] -= u_hot[gi]
        lanes_used += 1
    if rem.max() > 0:
        return None
    hottest = int(np.argmax(counts))
    for gi in range(G):
        while len(hot_blocks[gi]) < P:
            hot_blocks[gi].append(hottest)

    # lane lookup for hot blocks: block g, slot s -> lanes
    lane_of_hot = [dict() for _ in range(G)]
    for gi in range(G):
        for p, s in enumerate(hot_blocks[gi]):
            lane_of_hot[gi].setdefault(s, []).append(p)

    order = np.argsort(idx_c, kind="stable").astype(np.int64)
    starts = np.concatenate([[0], np.cumsum(counts)[:-1]])
    offs = np.full((P, n_instr), OOB, dtype=np.int32)
    for s in range(NUM_SLOTS):
        n_s = counts[s]
        if n_s == 0:
            continue
        cells = []
        for i in range(n_instr):
            b = seq[i]
            if b < 4:
                if LANE[s, b] >= 0:
                    cells.append((i, int(LANE[s, b])))
            else:
                for p in lane_of_hot[b - 4].get(s, []):
                    cells.append((i, p))
        if n_s > len(cells):
            return None
        sel = np.linspace(0, len(cells) - 1, n_s).round().astype(np.int64)
        toks = order[starts[s]:starts[s] + n_s]
        for t, ci in zip(toks, sel):
            i, p = cells[ci]
            offs[p, i] = t

    gflat = np.concatenate([np.asarray(hot_blocks[gi], np.int16) for gi in range(G)])
    gidx16 = np.zeros((16, NIDX // 16), dtype=np.int16)
    for i, s in enumerate(gflat):
        gidx16[i % 16, i // 16] = s
    gidx = np.tile(gidx16, (P // 16, 1))
    return offs, gidx


def _min_feasible_n(idx_c):
    n = max(G_START + 1, (TOK + P - 1) // P)
    while _design_core(idx_c, n) is None:
        n += 1
    return n


def _bf16_to_f32(a):
    return (a.view(np.uint16).astype(np.uint32) << np.uint32(16)).view(np.float32)


def _run(inputs, trace=False):
    import ml_dtypes

    ih = np.asarray(inputs["inputs_hour"])
    tb = np.ascontiguousarray(
        np.asarray(inputs["catid_time_matrix"], dtype=np.float32).astype(
            ml_dtypes.bfloat16
        )
    )
    idx_full = np.ascontiguousarray(ih.astype(np.int32).reshape(BATCH * SEQ))

    shards = [idx_full[c * TOK:(c + 1) * TOK] for c in range(N_CORES)]
    n_instr = max(_min_feasible_n(s) for s in shards)
    designs = [_design_core(s, n_instr) for s in shards]

    nc = _get_nc(n_instr)
    in_maps = [
        {
            "table": tb,
            "offs": np.ascontiguousarray(designs[c][0]),
            "gidx": np.ascontiguousarray(designs[c][1]),
        }
        for c in range(N_CORES)
    ]
    res = run_bass_kernel_spmd(nc, in_maps, core_ids=list(range(N_CORES)), trace=trace)
    outs = [
        _bf16_to_f32(np.asarray(res.results[i]["out"])).reshape(B_CORE, SEQ, NUM_CATS)
        for i in range(N_CORES)
    ]
    full = np.concatenate(outs, axis=0)
    return full, res


def kernel(**inputs):
    full, _ = _run(inputs, trace=False)
    return full
